# revision 40
# baseline (speedup 1.0000x reference)
"""Masked attention kernel for Trainium2, row-parallel over 8 NeuronCores.

Reference computation (per problem):
    S   = (Q @ K^T) / sqrt(D)          [NQ, NK]
    S   = where(m, S, -1e30)
    P   = softmax(S, axis=-1)
    out = P @ V                        [NQ, D]

Strategy:
  * Shard queries across 8 cores (1024 rows each); K/V/mask-columns replicated
    or sliced appropriately. No collectives.
  * Scores are computed TRANSPOSED on-chip: S_T[k, q] = sum_d K[k,d] * Qs[q,d]
    with Qs = Q/sqrt(D) pre-scaled on host. This makes the second matmul
    (P^T as lhsT, V as rhs) transpose-free.
  * Softmax without max-subtraction (scores are ~N(0,1); exp is safe in f32),
    multiplying by the 0/1 mask after exp.
  * Denominator for free: V is extended with a ones column, so PSUM column 256
    accumulates sum_k P[q,k]; normalize with reciprocal + per-partition scale.
  * bf16 matmul inputs (f32 PSUM accumulation); output DMA'd as bf16 and
    cast back to f32 on host (adds ~2e-4 rel err against a 2e-2 budget,
    halves the latency-critical final transfer).
"""

import os
import sys

import numpy as np

sys.path.insert(0, "/opt/trn_rl_repo")

import ml_dtypes

NQ, NK, D = 8192, 8192, 256
NCORES = 8
QSH = NQ // NCORES          # 1024 queries per core
P = 128
KT_TILES = NK // P          # 64 key tiles
QB = 512                    # q-block (matmul moving free dim)
NQB = QSH // QB             # 2 q-blocks per core
VE = D + 1                  # V extended with ones column

_STATE = {}
LAST_RESULTS = None
TRACE = bool(os.environ.get("BASS_TRACE"))


def _build():
    import concourse.tile as tile
    from concourse import bacc, mybir

    bf16 = mybir.dt.bfloat16
    f32 = mybir.dt.float32
    u8 = mybir.dt.uint8

    nc = bacc.Bacc("TRN2", debug=False, enable_asserts=False, num_devices=NCORES)

    # All big inputs are partition-major: [128, KT_TILES, free] so chunked
    # DMAs move large contiguous per-partition ranges.  K and V are packed
    # into ONE tensor per k-tile (K's 256 d-elems then V's 257 cols) so a
    # single ~600ns DMA trigger moves both streams per group.
    kv_d = nc.dram_tensor(
        "KV", [P, KT_TILES, 2 * P + VE], bf16, kind="ExternalInput"
    ).ap()
    qt_d = nc.dram_tensor("QT", [P, 2, QSH], bf16, kind="ExternalInput").ap()
    mt_d = nc.dram_tensor("MT", [P, KT_TILES, QSH], u8, kind="ExternalInput").ap()
    # Output rows are written qb-major, then 128-row chunk, then partition —
    # [NQB, 4, P, D] is dense row-major identical to [QSH, D].  bf16 halves
    # the final (latency-critical) DMA; host casts back to f32.
    out_d = nc.dram_tensor("out", [NQB, 4, P, D], bf16, kind="ExternalOutput").ap()

    Exp = mybir.ActivationFunctionType.Exp
    mult = mybir.AluOpType.mult

    with tile.TileContext(nc) as tc:
        with (
            tc.tile_pool(name="singles", bufs=1) as singles,
            tc.tile_pool(name="pp", bufs=8) as pp,
            tc.tile_pool(name="outp", bufs=3) as outp,
            tc.tile_pool(name="smallp", bufs=4) as smallp,
            tc.tile_pool(name="spsum", bufs=4, space="PSUM") as spsum,
            tc.tile_pool(name="opsum", bufs=1, space="PSUM") as opsum,
        ):
            # Prewarm the ACT exp table so its ~2.7us load overlaps input DMA.
            warm = singles.tile([P, 1], f32)
            nc.vector.memset(warm, 0.0)
            warm2 = smallp.tile([P, 1], f32, tag="warm2")
            nc.scalar.activation(warm2, warm, Exp)

            # Warm the PE HAM clock gate (~3.4us of matmul activity) while the
            # input DMAs stream in, so the real k-loop starts at 2.4 GHz.
            # The dummy matmuls read a raw (untracked, uninitialized) SBUF
            # tensor so they have no dependencies and start right after the
            # prologue barrier.
            wdummy = nc.alloc_sbuf_tensor("wdummy", [P, QB], bf16).ap()
            w_ps = spsum.tile([P, QB], f32, tag="s", name="w_ps")
            NWARM = 10
            for i in range(NWARM):
                nc.tensor.matmul(
                    w_ps, lhsT=wdummy[:, 0:P], rhs=wdummy,
                    start=(i == 0), stop=(i == NWARM - 1),
                )

            qt_sb = singles.tile([P, 2, QSH], bf16)
            kv_sb = singles.tile([P, KT_TILES, 2 * P + VE], bf16)
            mt_sb = singles.tile([P, KT_TILES, QSH], u8)

            # Chunked input DMAs, ordered by first-use time in the k-loop;
            # first chunks are small so the loop can start early.
            def kv_dma(a, b, eng=None):
                (eng or nc.sync).dma_start(
                    out=kv_sb[:, a:b, :], in_=kv_d[:, a:b, :]
                )

            def mt_dma(a, b, eng=None):
                (eng or nc.sync).dma_start(
                    out=mt_sb[:, a:b, :], in_=mt_d[:, a:b, :]
                )

            # Head: the first two tiles' worth of each stream goes out on
            # parallel rings (waits group per issuing engine) so mm1 t0 can
            # start at the aggregate-bandwidth floor (~11us).  Steady state:
            # one ring (sync), strict need order k,mask,v per group — proven
            # stall-free in the baseline.
            nc.sync.dma_start(out=qt_sb[:, :, 0:QB], in_=qt_d[:, :, 0:QB])
            kv_dma(0, 2, eng=nc.scalar)
            mt_dma(0, 2, eng=nc.gpsimd)
            groups = [(2, 5), (5, 9), (9, 15), (15, 23), (23, 33),
                      (33, 43), (43, 53), (53, 64)]
            for gi, (a, b) in enumerate(groups):
                kv_dma(a, b)
                mt_dma(a, b)
                if gi == 2:
                    # qb1 half of Q — not needed until the second pass.
                    nc.sync.dma_start(
                        out=qt_sb[:, :, QB:QSH], in_=qt_d[:, :, QB:QSH]
                    )

            # Software pipelining: mm1/exp/mask are issued SKEW k-tiles
            # ahead of mm2 in program order, so the PE never waits on the
            # ~1.9us mm1 -> exp -> mask-multiply chain (the scheduler's
            # implicit skew of ~2 tiles was marginal).
            # The previous q-block's epilogue is emitted two tiles into the
            # next q-block, so its DVE/ACT instructions queue BEHIND the new
            # block's first exp/mask ops instead of delaying them (which
            # stalled mm1 ~0.5us on PSUM recycling at the boundary).
            pending_epi = [None]

            def flush_epi():
                if pending_epi[0] is not None:
                    pending_epi[0]()
                    pending_epi[0] = None

            for qb in range(NQB):
                o_ps = [
                    opsum.tile([P, VE], f32, tag=f"o{qs}", name=f"o_ps{qs}")
                    for qs in range(4)
                ]
                p_tiles = {}

                def mm2_emit(j):
                    p_sb = p_tiles.pop(j)
                    for qs in range(4):
                        nc.tensor.matmul(
                            o_ps[qs],
                            lhsT=p_sb[:, qs * P:(qs + 1) * P],
                            rhs=kv_sb[:, j, 2 * P:2 * P + VE],
                            start=(j == 0),
                            stop=(j == KT_TILES - 1),
                        )

                j = 0
                for t in range(KT_TILES):
                    s_ps = spsum.tile([P, QB], f32, tag="s")
                    nc.tensor.matmul(
                        s_ps,
                        lhsT=kv_sb[:, t, 0:P],
                        rhs=qt_sb[:, 0, qb * QB:(qb + 1) * QB],
                        start=True,
                        stop=False,
                    )
                    nc.tensor.matmul(
                        s_ps,
                        lhsT=kv_sb[:, t, P:2 * P],
                        rhs=qt_sb[:, 1, qb * QB:(qb + 1) * QB],
                        start=False,
                        stop=True,
                    )
                    p_sb = pp.tile([P, QB], bf16, tag="p")
                    if t == KT_TILES - 1:
                        # Last tile: exp/mask in two half-width pieces so the
                        # end-of-loop dependency chain mm1->exp->mask->mm2 is
                        # ~0.7us shorter.
                        for h in range(2):
                            sl = slice(h * (QB // 2), (h + 1) * (QB // 2))
                            msl = slice(qb * QB + h * (QB // 2),
                                        qb * QB + (h + 1) * (QB // 2))
                            nc.scalar.activation(p_sb[:, sl], s_ps[:, sl], Exp)
                            nc.vector.tensor_tensor(
                                p_sb[:, sl], p_sb[:, sl], mt_sb[:, t, msl], mult
                            )
                    else:
                        nc.scalar.activation(p_sb, s_ps, Exp)
                        nc.vector.tensor_tensor(
                            p_sb, p_sb, mt_sb[:, t, qb * QB:(qb + 1) * QB], mult
                        )
                    p_tiles[t] = p_sb
                    if t == 1:
                        flush_epi()
                    skew = 3
                    while j <= t - skew:
                        mm2_emit(j)
                        j += 1
                        skew = 3
                while j < KT_TILES:
                    mm2_emit(j)
                    j += 1
                # Epilogue: normalize each 128-row chunk (DVE/ACT split the
                # four muls) and DMA it out.  Mid-kernel q-block: one merged
                # DMA, latency irrelevant, emission deferred via pending_epi.
                # Final q-block: latency is everything — emitted immediately,
                # and each chunk's DMA triggers as soon as its data is ready,
                # spread across rings.
                last_qb = qb == NQB - 1

                def epi(o_ps=o_ps, qb=qb, last_qb=last_qb):
                    o4_sb = outp.tile([P, 4, D], bf16, tag="o4")
                    # All reciprocals first: each o_ps[qs] denominator is
                    # ready as soon as its stop-matmul retires, and ACT's
                    # first mul only waits on recip[1] — not on DVE's first
                    # mul.
                    recips = []
                    for qs in range(4):
                        recip = smallp.tile([P, 1], f32, tag=f"recip{qs}")
                        nc.vector.reciprocal(recip, o_ps[qs][:, D:D + 1])
                        recips.append(recip)
                    for qs in range(4):
                        recip = recips[qs]
                        if qs % 2 == 0:
                            nc.vector.tensor_scalar_mul(
                                o4_sb[:, qs, :], o_ps[qs][:, 0:D], recip
                            )
                        else:
                            # ACT does the other half so the epilogue runs on
                            # two engines in parallel (GPSIMD can't read
                            # PSUM).
                            nc.scalar.mul(
                                o4_sb[:, qs, :], o_ps[qs][:, 0:D], recip
                            )
                        if last_qb:
                            # qs3's trigger issues from ACT right behind its
                            # own mul (no cross-engine sem hop on the
                            # critical path).
                            eng = [nc.sync, nc.gpsimd, nc.sync, nc.scalar][qs]
                            eng.dma_start(
                                out=out_d[qb, qs:qs + 1].rearrange(
                                    "c p d -> p c d"
                                ),
                                in_=o4_sb[:, qs:qs + 1, :],
                            )
                        elif qs == 3:
                            nc.gpsimd.dma_start(
                                out=out_d[qb].rearrange("c p d -> p c d"),
                                in_=o4_sb,
                            )

                if last_qb:
                    flush_epi()
                    epi()
                else:
                    pending_epi[0] = epi

    nc.compile()
    return nc


def _get_nc():
    if "nc" not in _STATE:
        _STATE["nc"] = _build()
    return _STATE["nc"]


def _prep_inputs(K, V, Q, m):
    bf16 = ml_dtypes.bfloat16
    scale = 1.0 / np.sqrt(np.float32(D))

    # KT[p, t, c*128+k] = K[t*128+k, c*128+p]   (p = d % 128, c = d // 128)
    kt = np.ascontiguousarray(
        K.astype(np.float32).reshape(KT_TILES, P, 2, P).transpose(3, 0, 2, 1)
    ).astype(bf16).reshape(P, KT_TILES, 2 * P)

    # VT[p, t, n] = V_ext[t*128+p, n]
    vt = np.ones((NK, VE), dtype=np.float32)
    vt[:, :D] = V
    vt = np.ascontiguousarray(
        vt.astype(bf16).reshape(KT_TILES, P, VE).transpose(1, 0, 2)
    )
    # One packed stream: per (p, t), K's 256 then V's 257 columns.
    kv = np.ascontiguousarray(np.concatenate([kt, vt], axis=2))

    # QT[p, c, q] = Q_scaled[q, c*128+p]  (per-core slice of q)
    qs_all = (Q.astype(np.float32) * scale).T.astype(bf16)  # [D, NQ]
    mt_all = np.ascontiguousarray(m.astype(np.uint8).T)     # [NK, NQ]

    in_maps = []
    for c in range(NCORES):
        q0 = c * QSH
        qt_c = np.ascontiguousarray(
            qs_all[:, q0:q0 + QSH].reshape(2, P, QSH).transpose(1, 0, 2)
        )
        # MT[p, t, q] = m[q0 + q, t*128 + p]
        mt_c = np.ascontiguousarray(
            mt_all[:, q0:q0 + QSH].reshape(KT_TILES, P, QSH).transpose(1, 0, 2)
        )
        in_maps.append({"KV": kv, "QT": qt_c, "MT": mt_c})
    return in_maps


def kernel(K, V, Q, m):
    global LAST_RESULTS
    from concourse.bass_utils import run_bass_kernel_spmd

    nc = _get_nc()
    in_maps = _prep_inputs(
        np.asarray(K), np.asarray(V), np.asarray(Q), np.asarray(m)
    )
    try:
        res = run_bass_kernel_spmd(
            nc, in_maps, core_ids=list(range(NCORES)), trace=TRACE
        )
    except Exception:
        # Profiling hook unavailable or a transient runtime failure — retry
        # once, untraced.
        os.environ.pop("BASS_TRACE", None)
        res = run_bass_kernel_spmd(
            nc, in_maps, core_ids=list(range(NCORES)), trace=False
        )
    LAST_RESULTS = res
    out = np.concatenate(
        [np.asarray(res.results[c]["out"]).reshape(QSH, D) for c in range(NCORES)],
        axis=0,
    )
    return out.astype(np.float32)



# revision 41
# speedup vs baseline: 1.0171x; 1.0171x over previous
"""Masked attention kernel for Trainium2, row-parallel over 8 NeuronCores.

Reference computation (per problem):
    S   = (Q @ K^T) / sqrt(D)          [NQ, NK]
    S   = where(m, S, -1e30)
    P   = softmax(S, axis=-1)
    out = P @ V                        [NQ, D]

Strategy:
  * Shard queries across 8 cores (1024 rows each); K/V/mask-columns replicated
    or sliced appropriately. No collectives.
  * Scores are computed TRANSPOSED on-chip: S_T[k, q] = sum_d K[k,d] * Qs[q,d]
    with Qs = Q/sqrt(D) pre-scaled on host. This makes the second matmul
    (P^T as lhsT, V as rhs) transpose-free.
  * Softmax without max-subtraction (scores are ~N(0,1); exp is safe in f32),
    multiplying by the 0/1 mask after exp.
  * Denominator for free: V is extended with a ones column, so PSUM column 256
    accumulates sum_k P[q,k]; normalize with reciprocal + per-partition scale.
  * bf16 matmul inputs (f32 PSUM accumulation); output DMA'd as bf16 and
    cast back to f32 on host (adds ~2e-4 rel err against a 2e-2 budget,
    halves the latency-critical final transfer).
"""

import os
import sys

import numpy as np

sys.path.insert(0, "/opt/trn_rl_repo")

import ml_dtypes

NQ, NK, D = 8192, 8192, 256
NCORES = 8
QSH = NQ // NCORES          # 1024 queries per core
P = 128
KT_TILES = NK // P          # 64 key tiles
QB = 512                    # q-block (matmul moving free dim)
NQB = QSH // QB             # 2 q-blocks per core
VE = D + 1                  # V extended with ones column

_STATE = {}
LAST_RESULTS = None
TRACE = bool(os.environ.get("BASS_TRACE"))


def _build():
    import concourse.tile as tile
    from concourse import bacc, mybir

    bf16 = mybir.dt.bfloat16
    f32 = mybir.dt.float32
    u8 = mybir.dt.uint8

    nc = bacc.Bacc("TRN2", debug=False, enable_asserts=False, num_devices=NCORES)

    # All big inputs are partition-major: [128, KT_TILES, free] so chunked
    # DMAs move large contiguous per-partition ranges.
    kt_d = nc.dram_tensor("KT", [P, KT_TILES, 2 * P], bf16, kind="ExternalInput").ap()
    vt_d = nc.dram_tensor("VT", [P, KT_TILES, VE], bf16, kind="ExternalInput").ap()
    qt_d = nc.dram_tensor("QT", [P, 2, QSH], bf16, kind="ExternalInput").ap()
    mt_d = nc.dram_tensor("MT", [P, KT_TILES, QSH], u8, kind="ExternalInput").ap()
    # Output rows are written qb-major, then 128-row chunk, then partition —
    # [NQB, 4, P, D] is dense row-major identical to [QSH, D].  bf16 halves
    # the final (latency-critical) DMA; host casts back to f32.
    out_d = nc.dram_tensor("out", [NQB, 4, P, D], bf16, kind="ExternalOutput").ap()

    Exp = mybir.ActivationFunctionType.Exp
    mult = mybir.AluOpType.mult

    with tile.TileContext(nc) as tc:
        with (
            tc.tile_pool(name="singles", bufs=1) as singles,
            tc.tile_pool(name="pp", bufs=8) as pp,
            tc.tile_pool(name="outp", bufs=3) as outp,
            tc.tile_pool(name="smallp", bufs=4) as smallp,
            tc.tile_pool(name="spsum", bufs=4, space="PSUM") as spsum,
            tc.tile_pool(name="opsum", bufs=1, space="PSUM") as opsum,
        ):
            # Prewarm the ACT exp table so its ~2.7us load overlaps input DMA.
            warm = singles.tile([P, 1], f32)
            nc.vector.memset(warm, 0.0)
            warm2 = smallp.tile([P, 1], f32, tag="warm2")
            nc.scalar.activation(warm2, warm, Exp)

            # Warm the PE HAM clock gate (~3.4us of matmul activity) while the
            # input DMAs stream in, so the real k-loop starts at 2.4 GHz.
            # The dummy matmuls read a raw (untracked, uninitialized) SBUF
            # tensor so they have no dependencies and start right after the
            # prologue barrier.
            wdummy = nc.alloc_sbuf_tensor("wdummy", [P, QB], bf16).ap()
            w_ps = spsum.tile([P, QB], f32, tag="s", name="w_ps")
            NWARM = 10
            for i in range(NWARM):
                nc.tensor.matmul(
                    w_ps, lhsT=wdummy[:, 0:P], rhs=wdummy,
                    start=(i == 0), stop=(i == NWARM - 1),
                )

            qt_sb = singles.tile([P, 2, QSH], bf16)
            kt_sb = singles.tile([P, KT_TILES, 2 * P], bf16)
            vt_sb = singles.tile([P, KT_TILES, VE], bf16)
            mt_sb = singles.tile([P, KT_TILES, QSH], u8)

            # Chunked input DMAs, ordered by first-use time in the k-loop;
            # first chunks are small so the loop can start early.
            def kt_dma(a, b, eng=None):
                (eng or nc.sync).dma_start(
                    out=kt_sb[:, a:b, :], in_=kt_d[:, a:b, :]
                )

            def vt_dma(a, b, eng=None):
                (eng or nc.sync).dma_start(
                    out=vt_sb[:, a:b, :], in_=vt_d[:, a:b, :]
                )

            def mt_dma(a, b, eng=None):
                (eng or nc.sync).dma_start(
                    out=mt_sb[:, a:b, :], in_=mt_d[:, a:b, :]
                )

            # Head: the first two tiles' worth of each stream goes out on
            # parallel rings (waits group per issuing engine) so mm1 t0 can
            # start at the aggregate-bandwidth floor (~11us).  Steady state:
            # one ring (sync), strict need order k,mask,v per group — proven
            # stall-free in the baseline.
            nc.sync.dma_start(out=qt_sb[:, :, 0:QB], in_=qt_d[:, :, 0:QB])
            kt_dma(0, 2, eng=nc.scalar)
            mt_dma(0, 2, eng=nc.gpsimd)
            vt_dma(0, 2)
            groups = [(2, 5), (5, 9), (9, 15), (15, 23), (23, 33),
                      (33, 43), (43, 53), (53, 64)]
            for gi, (a, b) in enumerate(groups):
                kt_dma(a, b)
                mt_dma(a, b)
                vt_dma(a, b)
                if gi == 2:
                    # qb1 half of Q — not needed until the second pass.
                    nc.sync.dma_start(
                        out=qt_sb[:, :, QB:QSH], in_=qt_d[:, :, QB:QSH]
                    )

            # Software pipelining: mm1/exp/mask are issued SKEW k-tiles
            # ahead of mm2 in program order, so the PE never waits on the
            # ~1.9us mm1 -> exp -> mask-multiply chain (the scheduler's
            # implicit skew of ~2 tiles was marginal).
            # The previous q-block's epilogue is emitted two tiles into the
            # next q-block, so its DVE/ACT instructions queue BEHIND the new
            # block's first exp/mask ops instead of delaying them (which
            # stalled mm1 ~0.5us on PSUM recycling at the boundary).
            pending_epi = [None]

            def flush_epi():
                if pending_epi[0] is not None:
                    pending_epi[0]()
                    pending_epi[0] = None

            for qb in range(NQB):
                o_ps = [
                    opsum.tile([P, VE], f32, tag=f"o{qs}", name=f"o_ps{qs}")
                    for qs in range(4)
                ]
                p_tiles = {}

                def mm2_emit(j):
                    p_sb = p_tiles.pop(j)
                    for qs in range(4):
                        nc.tensor.matmul(
                            o_ps[qs],
                            lhsT=p_sb[:, qs * P:(qs + 1) * P],
                            rhs=vt_sb[:, j, :],
                            start=(j == 0),
                            stop=(j == KT_TILES - 1),
                        )

                j = 0
                for t in range(KT_TILES):
                    s_ps = spsum.tile([P, QB], f32, tag="s")
                    nc.tensor.matmul(
                        s_ps,
                        lhsT=kt_sb[:, t, 0:P],
                        rhs=qt_sb[:, 0, qb * QB:(qb + 1) * QB],
                        start=True,
                        stop=False,
                    )
                    nc.tensor.matmul(
                        s_ps,
                        lhsT=kt_sb[:, t, P:2 * P],
                        rhs=qt_sb[:, 1, qb * QB:(qb + 1) * QB],
                        start=False,
                        stop=True,
                    )
                    p_sb = pp.tile([P, QB], bf16, tag="p")
                    if t == KT_TILES - 1:
                        # Last tile: exp/mask in two half-width pieces so the
                        # end-of-loop dependency chain mm1->exp->mask->mm2 is
                        # ~0.7us shorter.
                        for h in range(2):
                            sl = slice(h * (QB // 2), (h + 1) * (QB // 2))
                            msl = slice(qb * QB + h * (QB // 2),
                                        qb * QB + (h + 1) * (QB // 2))
                            nc.scalar.activation(p_sb[:, sl], s_ps[:, sl], Exp)
                            nc.vector.tensor_tensor(
                                p_sb[:, sl], p_sb[:, sl], mt_sb[:, t, msl], mult
                            )
                    else:
                        nc.scalar.activation(p_sb, s_ps, Exp)
                        nc.vector.tensor_tensor(
                            p_sb, p_sb, mt_sb[:, t, qb * QB:(qb + 1) * QB], mult
                        )
                    p_tiles[t] = p_sb
                    if t == 1:
                        flush_epi()
                    skew = 3
                    while j <= t - skew:
                        mm2_emit(j)
                        j += 1
                        skew = 3
                while j < KT_TILES:
                    mm2_emit(j)
                    j += 1
                # Epilogue: normalize each 128-row chunk (DVE/ACT split the
                # four muls) and DMA it out.  Mid-kernel q-block: one merged
                # DMA, latency irrelevant, emission deferred via pending_epi.
                # Final q-block: latency is everything — emitted immediately,
                # and each chunk's DMA triggers as soon as its data is ready,
                # spread across rings.
                last_qb = qb == NQB - 1

                def epi(o_ps=o_ps, qb=qb, last_qb=last_qb):
                    o4_sb = outp.tile([P, 4, D], bf16, tag="o4")
                    # All reciprocals first: each o_ps[qs] denominator is
                    # ready as soon as its stop-matmul retires, and ACT's
                    # first mul only waits on recip[1] — not on DVE's first
                    # mul.
                    recips = []
                    for qs in range(4):
                        recip = smallp.tile([P, 1], f32, tag=f"recip{qs}")
                        nc.vector.reciprocal(recip, o_ps[qs][:, D:D + 1])
                        recips.append(recip)
                    for qs in range(4):
                        recip = recips[qs]
                        if qs % 2 == 0:
                            nc.vector.tensor_scalar_mul(
                                o4_sb[:, qs, :], o_ps[qs][:, 0:D], recip
                            )
                        else:
                            # ACT does the other half so the epilogue runs on
                            # two engines in parallel (GPSIMD can't read
                            # PSUM).
                            nc.scalar.mul(
                                o4_sb[:, qs, :], o_ps[qs][:, 0:D], recip
                            )
                        if last_qb:
                            # qs3's trigger issues from ACT right behind its
                            # own mul (no cross-engine sem hop on the
                            # critical path).
                            eng = [nc.sync, nc.gpsimd, nc.sync, nc.scalar][qs]
                            eng.dma_start(
                                out=out_d[qb, qs:qs + 1].rearrange(
                                    "c p d -> p c d"
                                ),
                                in_=o4_sb[:, qs:qs + 1, :],
                            )
                        elif qs == 3:
                            nc.gpsimd.dma_start(
                                out=out_d[qb].rearrange("c p d -> p c d"),
                                in_=o4_sb,
                            )

                if last_qb:
                    flush_epi()
                    epi()
                else:
                    pending_epi[0] = epi

    nc.compile()
    return nc


def _get_nc():
    if "nc" not in _STATE:
        _STATE["nc"] = _build()
    return _STATE["nc"]


def _prep_inputs(K, V, Q, m):
    bf16 = ml_dtypes.bfloat16
    scale = 1.0 / np.sqrt(np.float32(D))

    # KT[p, t, c*128+k] = K[t*128+k, c*128+p]   (p = d % 128, c = d // 128)
    kt = np.ascontiguousarray(
        K.astype(np.float32).reshape(KT_TILES, P, 2, P).transpose(3, 0, 2, 1)
    ).astype(bf16).reshape(P, KT_TILES, 2 * P)

    # VT[p, t, n] = V_ext[t*128+p, n]
    vt = np.ones((NK, VE), dtype=np.float32)
    vt[:, :D] = V
    vt = np.ascontiguousarray(
        vt.astype(bf16).reshape(KT_TILES, P, VE).transpose(1, 0, 2)
    )

    # QT[p, c, q] = Q_scaled[q, c*128+p]  (per-core slice of q)
    qs_all = (Q.astype(np.float32) * scale).T.astype(bf16)  # [D, NQ]
    mt_all = np.ascontiguousarray(m.astype(np.uint8).T)     # [NK, NQ]

    in_maps = []
    for c in range(NCORES):
        q0 = c * QSH
        qt_c = np.ascontiguousarray(
            qs_all[:, q0:q0 + QSH].reshape(2, P, QSH).transpose(1, 0, 2)
        )
        # MT[p, t, q] = m[q0 + q, t*128 + p]
        mt_c = np.ascontiguousarray(
            mt_all[:, q0:q0 + QSH].reshape(KT_TILES, P, QSH).transpose(1, 0, 2)
        )
        in_maps.append({"KT": kt, "VT": vt, "QT": qt_c, "MT": mt_c})
    return in_maps


def kernel(K, V, Q, m):
    global LAST_RESULTS
    from concourse.bass_utils import run_bass_kernel_spmd

    nc = _get_nc()
    in_maps = _prep_inputs(
        np.asarray(K), np.asarray(V), np.asarray(Q), np.asarray(m)
    )
    try:
        res = run_bass_kernel_spmd(
            nc, in_maps, core_ids=list(range(NCORES)), trace=TRACE
        )
    except Exception:
        # Profiling hook unavailable or a transient runtime failure — retry
        # once, untraced.
        os.environ.pop("BASS_TRACE", None)
        res = run_bass_kernel_spmd(
            nc, in_maps, core_ids=list(range(NCORES)), trace=False
        )
    LAST_RESULTS = res
    out = np.concatenate(
        [np.asarray(res.results[c]["out"]).reshape(QSH, D) for c in range(NCORES)],
        axis=0,
    )
    return out.astype(np.float32)

